# revision 1
# baseline (speedup 1.0000x reference)
"""Trainium2 Bass kernel for the NCE cosine-similarity loss.

Problem: x [65536, 1024] f32 viewed as 1024 batches x 64 rows (1 orig, 8 pos,
55 neg). Per batch: cos(orig,pos_i) and cos(pos_i,neg_j), logits/0.1,
loss = logsumexp([cp, cn_*]) - cp, mean over all (batch, pos).

Strategy (8 NeuronCores, data-parallel over batches, 128 batches/core):
 - Two batches share one 128-partition tile ("pair"): rows on partitions.
 - DMA cast-load fp32->bf16 (SWDGE), then one xbar DMA-transpose per group of
   8 pairs gives [d-chunk, row] layout.
 - Per pair: 64x64-per-batch Gram (as one 128x128 via 8 accumulating bf16
   matmuls over the 8 d-chunks of 128). Diagonal -> norms. inv = sqrt(10)/norm
   (folds the 1/tau=10 logit scale into both cosine normalizations).
 - Column scale (with excluded pos-pos columns zeroed -> exp()=1, subtract 8
   later) via a K=1 broadcast matmul + DVE multiply; row scale fused into
   ScalarE Exp activation which also accumulates the per-row sum.
 - loss row p: log(sum - 8) - l0. Per-core partial sums [128, 4] go to HBM;
   host combines the 8 cores and divides by 8192.
"""

import sys

if "/opt/trn_rl_repo" not in sys.path:
    sys.path.insert(0, "/opt/trn_rl_repo")

import numpy as np

N_CORES = 8
ROWS_PER_CORE = 8192          # 128 batches x 64 rows
D = 1024
N_GROUPS = 8                  # dma groups of 8 pairs per core
N_QUADS = 16                  # quads of 4 pairs per core
N_PAIRS = 64                  # 2 batches per pair

_CACHE = {}


def _build(repeat=1, loop_n=0, stage=8, dma_once=False):
    import concourse.bacc as bacc
    import concourse.mybir as mybir
    import concourse.tile as tile

    dt = mybir.dt
    AF = mybir.ActivationFunctionType
    ALU = mybir.AluOpType

    nc = bacc.Bacc("TRN2", target_bir_lowering=False, debug=False, num_devices=N_CORES)
    x = nc.dram_tensor("x", [ROWS_PER_CORE, D], dt.float32, kind="ExternalInput")
    identb_d = nc.dram_tensor("identb", [128, 128], dt.bfloat16, kind="ExternalInput")
    mask_d = nc.dram_tensor("mask", [4, 128], dt.bfloat16, kind="ExternalInput")
    sel_d = nc.dram_tensor("sel", [4, 512], dt.bfloat16, kind="ExternalInput")
    neg8_d = nc.dram_tensor("neg8", [128, 1], dt.float32, kind="ExternalInput")
    out_d = nc.dram_tensor("out", [128, 4], dt.float32, kind="ExternalOutput")

    # x rows (g j p) d: group g, pair-in-group j, partition p
    xg = x.rearrange("(g j p) d -> g p j d", g=N_GROUPS, j=8, p=128)

    with tile.TileContext(nc) as tc:
        from contextlib import ExitStack

        with ExitStack() as ctx:
            cpool = ctx.enter_context(tc.tile_pool(name="consts", bufs=1))
            rowp = ctx.enter_context(tc.tile_pool(name="row", bufs=2))
            tp = ctx.enter_context(tc.tile_pool(name="tgrp", bufs=6))
            tpsp = ctx.enter_context(tc.tile_pool(name="tps", bufs=4, space="PSUM"))
            gramp = ctx.enter_context(tc.tile_pool(name="gram", bufs=2, space="PSUM"))
            bcpsp = ctx.enter_context(tc.tile_pool(name="bcps", bufs=2, space="PSUM"))
            gsbp = ctx.enter_context(tc.tile_pool(name="gsb", bufs=10))
            sb = ctx.enter_context(tc.tile_pool(name="sb", bufs=2))
            t2p = ctx.enter_context(tc.tile_pool(name="t2", bufs=3))
            stg = ctx.enter_context(tc.tile_pool(name="stg", bufs=1))

            identb = cpool.tile([128, 128], dt.bfloat16)
            nc.sync.dma_start(out=identb[:], in_=identb_d[:])
            maskc = cpool.tile([4, 128], dt.bfloat16)
            nc.sync.dma_start(out=maskc[:], in_=mask_d[:])
            selc = cpool.tile([4, 512], dt.bfloat16)
            nc.sync.dma_start(out=selc[:], in_=sel_d[:])
            neg8c = cpool.tile([128, 1], dt.float32)
            nc.sync.dma_start(out=neg8c[:], in_=neg8_d[:])

            s_stage = stg.tile([128, 2, 64], dt.float32, tag="s_stage")
            l0_stage = stg.tile([128, 2, 64], dt.float32, tag="l0_stage")

            from contextlib import nullcontext

            def phase_a(q, row_tiles):
                """loads, transposes, grams, diag sums for quad q"""
                if q % 2 == 0 and (not dma_once or not row_tiles):
                    G = 0 if dma_once else q // 2
                    row = rowp.tile([128, 8, D], dt.bfloat16, tag="row")
                    nc.gpsimd.dma_start(out=row[:], in_=xg[G])
                    row_tiles[G] = row
                row = row_tiles[0 if dma_once else q // 2]

                n2q = sb.tile([128, 4], dt.float32, tag="n2q")
                grams = []
                t_list = []
                for jj in range(4):
                    j_in_g = (q % 2) * 4 + jj
                    g_abs = 4 * q + jj
                    tps = tpsp.tile([128, 8, 128], dt.bfloat16, tag="tps")
                    for c in range(8):
                        nc.tensor.transpose(
                            tps[:, c, :],
                            row[:, j_in_g, c * 128 : (c + 1) * 128],
                            identb[:],
                        )
                    t = tp.tile([128, 8, 128], dt.bfloat16, tag="t")
                    if g_abs % 2 == 0:
                        nc.vector.tensor_copy(t.rearrange("p a b -> p (a b)"),
                                              tps.rearrange("p a b -> p (a b)"))
                    else:
                        nc.scalar.copy(t.rearrange("p a b -> p (a b)"),
                                       tps.rearrange("p a b -> p (a b)"))
                    t_list.append(t)
                if stage < 2:
                    return n2q, grams
                for jj in range(4):
                    g_abs = 4 * q + jj
                    t = t_list[jj]
                    gps = gramp.tile([128, 128], dt.float32, tag="gram")
                    for c in range(8):
                        nc.tensor.matmul(
                            gps[:],
                            t[:, c, :],
                            t[:, c, :],
                            start=(c == 0),
                            stop=(c == 7),
                        )
                    gsb = gsbp.tile([128, 128], dt.bfloat16, tag="gsb")
                    if g_abs % 2 == 0:
                        nc.scalar.copy(gsb[:], gps[:])
                    else:
                        nc.vector.tensor_copy(gsb[:], gps[:])
                    scr = sb.tile([128, 128], dt.bfloat16, tag="scr")
                    nc.vector.tensor_mul(scr[:], gsb[:], identb[:])
                    nc.vector.reduce_sum(
                        n2q[:, jj : jj + 1], scr[:], axis=mybir.AxisListType.X
                    )
                    grams.append(gsb)
                return n2q, grams

            def phase_b(q, n2q, grams):
                """normalization + exp/log-sum staging for quad q"""
                if stage < 4:
                    return
                rcpq = sb.tile([128, 4], dt.float32, tag="rcpq")
                nc.vector.reciprocal(rcpq[:], n2q[:])
                invq = sb.tile([128, 4], dt.float32, tag="invq")
                # inv = sqrt(10/n2): folds tau and both norm factors
                nc.scalar.activation(invq[:], rcpq[:], AF.Sqrt, scale=10.0)

                if stage < 5:
                    return
                invb = sb.tile([128, 4], dt.bfloat16, tag="invb")
                nc.vector.tensor_copy(invb[:], invq[:])
                itps = bcpsp.tile([4, 128], dt.float32, tag="bcps")
                nc.tensor.matmul(itps[:], invb[:], identb[:])
                itsb = sb.tile([4, 128], dt.bfloat16, tag="itsb")
                nc.vector.tensor_mul(itsb[:], itps[:], maskc[:])

                if stage < 6:
                    return
                t2q = t2p.tile([128, 4, 128], dt.float32, tag="t2")
                for jj in range(4):
                    bcps = bcpsp.tile([128, 128], dt.float32, tag="bcps")
                    nc.tensor.matmul(
                        bcps[:], selc[:, jj * 128 : (jj + 1) * 128], itsb[:]
                    )
                    # fully-scaled logits: (G * inv[p]) * (inv_masked[j])
                    nc.vector.scalar_tensor_tensor(
                        t2q[:, jj, :],
                        grams[jj][:],
                        invq[:, jj : jj + 1],
                        bcps[:],
                        op0=ALU.mult,
                        op1=ALU.mult,
                    )
                if stage < 7:
                    return
                escr = sb.tile([128, 4, 128], dt.float32, tag="escr")
                nc.scalar.activation(
                    escr.rearrange("p a b -> p (a b)"),
                    t2q.rearrange("p a b -> p (a b)"),
                    AF.Exp,
                )
                nc.vector.reduce_sum(
                    s_stage[:, :, 4 * q : 4 * q + 4].rearrange("p h g -> p g h"),
                    escr.rearrange("p a (c h) -> p (a c) h", h=64),
                    axis=mybir.AxisListType.X,
                )
                nc.vector.tensor_copy(
                    l0_stage[:, :, 4 * q : 4 * q + 4].rearrange("p h g -> p g h"),
                    t2q.rearrange("p a (c h) -> p a c", h=64)
                    if False
                    else t2q.rearrange("p a b -> p (a b)")[:, 0:512:64].rearrange(
                        "p (g h) -> p g h", h=2
                    ),
                )

            loop_cm = tc.For_i(0, loop_n, 1) if loop_n else nullcontext()
            with loop_cm:
                row_tiles = {}
                pending = None
                for q in range(N_QUADS * repeat):
                    q = q % N_QUADS
                    state = phase_a(q, row_tiles)
                    if pending is not None:
                        phase_b(pending[0], pending[1], pending[2])
                    pending = (q, *state)
                if pending is not None:
                    phase_b(pending[0], pending[1], pending[2])

            final = stg.tile([128, 4], dt.float32, tag="final")
            if stage >= 7:
                lnout = stg.tile([128, 2, 64], dt.float32, tag="lnout")
                nc.scalar.activation(
                    lnout.rearrange("p a b -> p (a b)"),
                    s_stage.rearrange("p a b -> p (a b)"),
                    AF.Ln,
                    bias=neg8c[:],
                )
                nc.vector.reduce_sum(
                    final[:, 0:2], lnout[:], axis=mybir.AxisListType.X
                )
                nc.vector.reduce_sum(
                    final[:, 2:4], l0_stage[:], axis=mybir.AxisListType.X
                )
            else:
                nc.vector.memset(final[:], 0.0)
                nc.vector.memset(s_stage.rearrange("p a b -> p (a b)"), 0.0)
                nc.vector.memset(l0_stage.rearrange("p a b -> p (a b)"), 0.0)
            nc.gpsimd.dma_start(out=out_d[:], in_=final[:])

    nc.compile()
    return nc


def _consts():
    import ml_dtypes

    bf = ml_dtypes.bfloat16
    ident = np.eye(128, dtype=np.float32)  # unused on device now
    mask = np.ones((4, 128), dtype=bf)
    mask[:, 1:9] = 0.0
    mask[:, 65:73] = 0.0
    sel = np.zeros((4, 512), dtype=bf)
    for jj in range(4):
        sel[jj, jj * 128 : (jj + 1) * 128] = 1.0
    neg8 = np.full((128, 1), -8.0, dtype=np.float32)
    identb = np.eye(128, dtype=bf)
    return ident, mask, sel, neg8, identb


def kernel(x, labels=None, **_unused):
    from concourse.bass_utils import run_bass_kernel_spmd

    x = np.ascontiguousarray(np.asarray(x, dtype=np.float32))
    assert x.shape == (N_CORES * ROWS_PER_CORE, D), x.shape

    if "nc" not in _CACHE:
        _CACHE["nc"] = _build()
    nc = _CACHE["nc"]

    ident, mask, sel, neg8, identb = _consts()
    in_maps = [
        {
            "x": x[i * ROWS_PER_CORE : (i + 1) * ROWS_PER_CORE],
            "mask": mask,
            "sel": sel,
            "neg8": neg8,
            "identb": identb,
        }
        for i in range(N_CORES)
    ]
    res = run_bass_kernel_spmd(nc, in_maps, list(range(N_CORES)))

    total = 0.0
    for r in res.results:
        o = r["out"].astype(np.float64)
        # valid rows: pos rows of batch A (partitions 1..8, half A) and of
        # batch B (partitions 65..72, half B)
        total += o[1:9, 0].sum() - o[1:9, 2].sum()
        total += o[65:73, 1].sum() - o[65:73, 3].sum()
    loss = total / (1024 * 8)
    return np.array(loss, dtype=np.float32)



# revision 2
# speedup vs baseline: 1.4198x; 1.4198x over previous
"""Trainium2 Bass kernel for the NCE cosine-similarity loss.

Problem: x [65536, 1024] f32 viewed as 1024 batches x 64 rows (1 orig, 8 pos,
55 neg). Per batch: cos(orig,pos_i) and cos(pos_i,neg_j), logits/0.1,
loss = logsumexp([cp, cn_*]) - cp, mean over all (batch, pos).

Strategy (8 NeuronCores, data-parallel over batches, 128 batches/core):
 - Two batches share one 128-partition tile ("pair"): rows on partitions.
 - DMA cast-load fp32->bf16 (SWDGE), then one xbar DMA-transpose per group of
   8 pairs gives [d-chunk, row] layout.
 - Per pair: 64x64-per-batch Gram (as one 128x128 via 8 accumulating bf16
   matmuls over the 8 d-chunks of 128). Diagonal -> norms. inv = sqrt(10)/norm
   (folds the 1/tau=10 logit scale into both cosine normalizations).
 - Column scale (with excluded pos-pos columns zeroed -> exp()=1, subtract 8
   later) via a K=1 broadcast matmul + DVE multiply; row scale fused into
   ScalarE Exp activation which also accumulates the per-row sum.
 - loss row p: log(sum - 8) - l0. Per-core partial sums [128, 4] go to HBM;
   host combines the 8 cores and divides by 8192.
"""

import sys

if "/opt/trn_rl_repo" not in sys.path:
    sys.path.insert(0, "/opt/trn_rl_repo")

import numpy as np

N_CORES = 8
ROWS_PER_CORE = 8192          # 128 batches x 64 rows
D = 1024
N_GROUPS = 8                  # dma groups of 8 pairs per core
N_QUADS = 16                  # quads of 4 pairs per core
N_PAIRS = 64                  # 2 batches per pair

_CACHE = {}


def _build(repeat=1, loop_n=0, stage=8, dma_once=False):
    import concourse.bacc as bacc
    import concourse.mybir as mybir
    import concourse.tile as tile

    dt = mybir.dt
    AF = mybir.ActivationFunctionType
    ALU = mybir.AluOpType

    nc = bacc.Bacc("TRN2", target_bir_lowering=False, debug=False, num_devices=N_CORES)
    x = nc.dram_tensor("x", [ROWS_PER_CORE, D], dt.float32, kind="ExternalInput")
    identb_d = nc.dram_tensor("identb", [128, 128], dt.bfloat16, kind="ExternalInput")
    mask_d = nc.dram_tensor("mask", [4, 128], dt.bfloat16, kind="ExternalInput")
    sel_d = nc.dram_tensor("sel", [4, 512], dt.bfloat16, kind="ExternalInput")
    neg8_d = nc.dram_tensor("neg8", [128, 1], dt.float32, kind="ExternalInput")
    out_d = nc.dram_tensor("out", [128, 4], dt.float32, kind="ExternalOutput")

    # x rows (g j p) d: group g, pair-in-group j, partition p
    xg = x.rearrange("(g j p) d -> g p j d", g=N_GROUPS, j=8, p=128)

    with tile.TileContext(nc) as tc:
        from contextlib import ExitStack

        with ExitStack() as ctx:
            cpool = ctx.enter_context(tc.tile_pool(name="consts", bufs=1))
            rowp = ctx.enter_context(tc.tile_pool(name="row", bufs=2))
            tp = ctx.enter_context(tc.tile_pool(name="tgrp", bufs=6))
            tpsp = ctx.enter_context(tc.tile_pool(name="tps", bufs=4, space="PSUM"))
            gramp = ctx.enter_context(tc.tile_pool(name="gram", bufs=2, space="PSUM"))
            bcpsp = ctx.enter_context(tc.tile_pool(name="bcps", bufs=2, space="PSUM"))
            gsbp = ctx.enter_context(tc.tile_pool(name="gsb", bufs=10))
            sb = ctx.enter_context(tc.tile_pool(name="sb", bufs=2))
            t2p = ctx.enter_context(tc.tile_pool(name="t2", bufs=3))
            stg = ctx.enter_context(tc.tile_pool(name="stg", bufs=1))

            identb = cpool.tile([128, 128], dt.bfloat16)
            nc.sync.dma_start(out=identb[:], in_=identb_d[:])
            maskc = cpool.tile([4, 128], dt.bfloat16)
            nc.sync.dma_start(out=maskc[:], in_=mask_d[:])
            selc = cpool.tile([4, 512], dt.bfloat16)
            nc.sync.dma_start(out=selc[:], in_=sel_d[:])
            neg8c = cpool.tile([128, 1], dt.float32)
            nc.sync.dma_start(out=neg8c[:], in_=neg8_d[:])

            s_stage = stg.tile([128, 2, 64], dt.float32, tag="s_stage")
            l0_stage = stg.tile([128, 2, 64], dt.float32, tag="l0_stage")

            from contextlib import nullcontext

            def phase_a(q, row_tiles):
                """loads, transposes, grams, diag sums for quad q"""
                if q % 2 == 0 and (not dma_once or not row_tiles):
                    G = 0 if dma_once else q // 2
                    row = rowp.tile([128, 8, D], dt.bfloat16, tag="row")
                    nc.gpsimd.dma_start(out=row[:], in_=xg[G])
                    row_tiles[G] = row
                row = row_tiles[0 if dma_once else q // 2]

                n2q = sb.tile([128, 4], dt.float32, tag="n2q")
                grams = []
                t_list = []
                for jj in range(4):
                    j_in_g = (q % 2) * 4 + jj
                    g_abs = 4 * q + jj
                    tps = tpsp.tile([128, 8, 128], dt.bfloat16, tag="tps")
                    for c in range(8):
                        nc.tensor.transpose(
                            tps[:, c, :],
                            row[:, j_in_g, c * 128 : (c + 1) * 128],
                            identb[:],
                        )
                    t = tp.tile([128, 8, 128], dt.bfloat16, tag="t")
                    if g_abs % 2 == 0:
                        nc.vector.tensor_copy(t.rearrange("p a b -> p (a b)"),
                                              tps.rearrange("p a b -> p (a b)"))
                    else:
                        nc.scalar.copy(t.rearrange("p a b -> p (a b)"),
                                       tps.rearrange("p a b -> p (a b)"))
                    t_list.append(t)
                if stage < 2:
                    return n2q, grams
                for jj in range(4):
                    g_abs = 4 * q + jj
                    t = t_list[jj]
                    gps = gramp.tile([128, 128], dt.float32, tag="gram")
                    for c in range(8):
                        nc.tensor.matmul(
                            gps[:],
                            t[:, c, :],
                            t[:, c, :],
                            start=(c == 0),
                            stop=(c == 7),
                        )
                    gsb = gsbp.tile([128, 128], dt.bfloat16, tag="gsb")
                    if g_abs % 2 == 0:
                        nc.scalar.copy(gsb[:], gps[:])
                    else:
                        nc.vector.tensor_copy(gsb[:], gps[:])
                    scr = sb.tile([128, 128], dt.bfloat16, tag="scr")
                    nc.vector.tensor_mul(scr[:], gsb[:], identb[:])
                    nc.vector.reduce_sum(
                        n2q[:, jj : jj + 1], scr[:], axis=mybir.AxisListType.X
                    )
                    grams.append(gsb)
                return n2q, grams

            def phase_b(q, n2q, grams):
                """normalization + exp/log-sum staging for quad q"""
                if stage < 4:
                    return
                # inv = sqrt(10/n2) = exp(-0.5*ln(0.1*n2)): Ln/Exp share an
                # ACT table set with Copy, avoiding per-quad table reloads
                lnn2 = sb.tile([128, 4], dt.float32, tag="lnn2")
                nc.scalar.activation(lnn2[:], n2q[:], AF.Ln, scale=0.1)
                invq = sb.tile([128, 4], dt.float32, tag="invq")
                nc.scalar.activation(invq[:], lnn2[:], AF.Exp, scale=-0.5)

                if stage < 5:
                    return
                invb = sb.tile([128, 4], dt.bfloat16, tag="invb")
                nc.vector.tensor_copy(invb[:], invq[:])
                itps = bcpsp.tile([4, 128], dt.float32, tag="bcps")
                nc.tensor.matmul(itps[:], invb[:], identb[:])
                itsb = sb.tile([4, 128], dt.bfloat16, tag="itsb")
                nc.vector.tensor_mul(itsb[:], itps[:], maskc[:])

                if stage < 6:
                    return
                t2q = t2p.tile([128, 4, 128], dt.float32, tag="t2")
                for jj in range(4):
                    bcps = bcpsp.tile([128, 128], dt.float32, tag="bcps")
                    nc.tensor.matmul(
                        bcps[:], selc[:, jj * 128 : (jj + 1) * 128], itsb[:]
                    )
                    # fully-scaled logits: (G * inv[p]) * (inv_masked[j])
                    nc.vector.scalar_tensor_tensor(
                        t2q[:, jj, :],
                        grams[jj][:],
                        invq[:, jj : jj + 1],
                        bcps[:],
                        op0=ALU.mult,
                        op1=ALU.mult,
                    )
                if stage < 7:
                    return
                escr = sb.tile([128, 4, 128], dt.float32, tag="escr")
                nc.scalar.activation(
                    escr.rearrange("p a b -> p (a b)"),
                    t2q.rearrange("p a b -> p (a b)"),
                    AF.Exp,
                )
                nc.vector.reduce_sum(
                    s_stage[:, :, 4 * q : 4 * q + 4].rearrange("p h g -> p g h"),
                    escr.rearrange("p a (c h) -> p (a c) h", h=64),
                    axis=mybir.AxisListType.X,
                )
                nc.vector.tensor_copy(
                    l0_stage[:, :, 4 * q : 4 * q + 4].rearrange("p h g -> p g h"),
                    t2q.rearrange("p a (c h) -> p a c", h=64)
                    if False
                    else t2q.rearrange("p a b -> p (a b)")[:, 0:512:64].rearrange(
                        "p (g h) -> p g h", h=2
                    ),
                )

            loop_cm = tc.For_i(0, loop_n, 1) if loop_n else nullcontext()
            with loop_cm:
                row_tiles = {}
                pending = None
                for q in range(N_QUADS * repeat):
                    q = q % N_QUADS
                    state = phase_a(q, row_tiles)
                    if pending is not None:
                        phase_b(pending[0], pending[1], pending[2])
                    pending = (q, *state)
                if pending is not None:
                    phase_b(pending[0], pending[1], pending[2])

            final = stg.tile([128, 4], dt.float32, tag="final")
            if stage >= 7:
                lnout = stg.tile([128, 2, 64], dt.float32, tag="lnout")
                nc.scalar.activation(
                    lnout.rearrange("p a b -> p (a b)"),
                    s_stage.rearrange("p a b -> p (a b)"),
                    AF.Ln,
                    bias=neg8c[:],
                )
                nc.vector.reduce_sum(
                    final[:, 0:2], lnout[:], axis=mybir.AxisListType.X
                )
                nc.vector.reduce_sum(
                    final[:, 2:4], l0_stage[:], axis=mybir.AxisListType.X
                )
            else:
                nc.vector.memset(final[:], 0.0)
                nc.vector.memset(s_stage.rearrange("p a b -> p (a b)"), 0.0)
                nc.vector.memset(l0_stage.rearrange("p a b -> p (a b)"), 0.0)
            nc.gpsimd.dma_start(out=out_d[:], in_=final[:])

    nc.compile()
    return nc


def _consts():
    import ml_dtypes

    bf = ml_dtypes.bfloat16
    ident = np.eye(128, dtype=np.float32)  # unused on device now
    mask = np.ones((4, 128), dtype=bf)
    mask[:, 1:9] = 0.0
    mask[:, 65:73] = 0.0
    sel = np.zeros((4, 512), dtype=bf)
    for jj in range(4):
        sel[jj, jj * 128 : (jj + 1) * 128] = 1.0
    neg8 = np.full((128, 1), -8.0, dtype=np.float32)
    identb = np.eye(128, dtype=bf)
    return ident, mask, sel, neg8, identb


def kernel(x, labels=None, **_unused):
    from concourse.bass_utils import run_bass_kernel_spmd

    x = np.ascontiguousarray(np.asarray(x, dtype=np.float32))
    assert x.shape == (N_CORES * ROWS_PER_CORE, D), x.shape

    if "nc" not in _CACHE:
        _CACHE["nc"] = _build()
    nc = _CACHE["nc"]

    ident, mask, sel, neg8, identb = _consts()
    in_maps = [
        {
            "x": x[i * ROWS_PER_CORE : (i + 1) * ROWS_PER_CORE],
            "mask": mask,
            "sel": sel,
            "neg8": neg8,
            "identb": identb,
        }
        for i in range(N_CORES)
    ]
    res = run_bass_kernel_spmd(nc, in_maps, list(range(N_CORES)))

    total = 0.0
    for r in res.results:
        o = r["out"].astype(np.float64)
        # valid rows: pos rows of batch A (partitions 1..8, half A) and of
        # batch B (partitions 65..72, half B)
        total += o[1:9, 0].sum() - o[1:9, 2].sum()
        total += o[65:73, 1].sum() - o[65:73, 3].sum()
    loss = total / (1024 * 8)
    return np.array(loss, dtype=np.float32)

